# revision 49
# baseline (speedup 1.0000x reference)
"""Trainium2 Bass kernel for the ADI diffusion layer.

The reference applies 10 ADI time steps to u[B=128, 1, 256, 256]; each step
does three tridiagonal (Thomas) solves along W or H with coefficients that
depend only on tiny [256] parameter vectors and the (compile-time-known)
step times.  The whole network is linear in u, and the x-axis solves
(right-multiplications) commute with the y-axis solves (left-
multiplications), so the entire computation collapses to

    out[b] = SY @ u[b] @ SX^T

with SX = product of the 20 x-solve inverses and SY = product of the 10
y-solve inverses, both 256x256, precomputed on host in float64 from the
parameter vectors.

On-device work per core (batch sharded 8 ways, 16 images/core):
  MM1: T1t = (SY @ u_b)^T  via  matmul(lhsT=u_b-tile, rhs=SY^T)
  MM2: out_b = T1t^T @ SX^T via matmul(lhsT=T1t-tile, rhs=SX^T)
Both stages contract on the partition dimension with the data tile as the
stationary operand, so the output lands in natural layout with zero
transposes.  The kernel is memory-bound: everything (u, SY^T, SX^T, the
intermediate, and the output) is carried in float16, which keeps the
end-to-end relative error ~4e-4 (measured) while halving HBM traffic vs
fp32 and running the PE at 1 cycle/row instead of 4.

SX and SY decay geometrically off the diagonal (per-step coeff <= ~5e-3),
so each 128-row contraction tile only feeds output columns within BAND of
its own index range ('banded2' matmuls: the overlap region accumulates via
per-element PSUM has_written, the rest overwrites; HW-verified).

DRAM layouts are partition-major/flat ([p, free]) so every DMA descriptor
covers a multi-KB contiguous run per partition; the matrices ship only
their band-packed 136-column windows.  Images are processed in groups of 2;
both stages of a group SHARE one 2-bank PSUM tile (MM2 overwrites the banks
MM1 used — its wait-after-read on the t1 evacuation is the existing chain
dependency, so sharing adds no serialization), which halves the per-group
PSUM footprint and lets the single-tag pool hold 4 in-flight groups in the
8 banks.  That takes the group-recycle chain off the critical path: the
middle phase runs at the ACT/DVE PSUM-evacuation capacity (single 1024-elem
copies, the largest the banks allow, on DEDICATED streams: all t1 copies on
DVE, all ot copies on ACT, so the supply-paced t1 stream never queues
behind a chain-lagged ot copy on the same engine).  The
PE emission is software-pipelined (MM1 of group g+1 before MM2 of group g;
measured: deeper prologues delay the ot evacuations and grow the tail).  Chunk 0 (matrices +
group 0) is split across the SP and ACT HWDGE queues so compute starts
~0.6us earlier; the rest streams sequentially on SP (packet dispatch is a
global ~90/us, so sequential FIFO on one queue is optimal for a linear
consumer).  Outputs ride the SP queue (free of input by the time outputs
start; HWDGE keeps the postamble DGE drain short), except the final
group's DMA which ACT issues right after its own evacuation copy to skip
a cross-engine semaphore hop on the critical tail.

Walrus enforces tiny sync-wait-slot budgets (1 for matmuls, ACT/DVE copies
and DMACopies) that Tile's scheduler does not know about;
_fix_wait_limits() post-processes the scheduled BIR to drop transitively
implied waits and relocate the rest onto earlier same-engine instructions.
"""

import numpy as np

import concourse.bass as bass
import concourse.mybir as mybir
import concourse.tile as tile
from concourse.bass_utils import run_bass_kernel_spmd

SIZE = 256
B_FULL = 128
N_CORES = 8
B_PER = B_FULL // N_CORES  # 16 images per core
G = B_PER * 2              # 32 [128, 256] partition-tiles of u per core
P = 128

DT = 0.01
DX = 1.0
DY = 1.0
NUM_STEPS = 10
EPS = 1e-6

F32 = mybir.dt.float32
F16 = mybir.dt.float16
BAND = 8
PSUM_BUFS = 4


def _smooth32(v):
    vp = np.concatenate([v[:1], v, v[-1:]]).astype(np.float32)
    return (np.float32(0.25) * vp[:-2] + np.float32(0.5) * vp[1:-1]
            + np.float32(0.25) * vp[2:]).astype(np.float32)


def _coeffs_at32(base, lin, quad, t):
    t = np.float32(t)
    return np.maximum(base + lin * t + quad * (t * t), np.float32(EPS)).astype(np.float32)


def _solve_inv64(alpha_vec32, dt, dh):
    """Inverse of the tridiagonal system the reference's _diffuse solves.

    Coefficient construction mirrors the reference in float32; the inverse
    itself is taken in float64.
    """
    coeff = (_smooth32(alpha_vec32) * np.float32(dt) / np.float32(dh * dh)).astype(np.float32)
    a = (-coeff).astype(np.float64)
    c = (-coeff).astype(np.float64)
    b = (np.float32(1.0) + np.float32(2.0) * coeff).astype(np.float32).astype(np.float64)
    b[0] = np.float64(np.float32(1.0) + coeff[0])
    b[-1] = np.float64(np.float32(1.0) + coeff[-1])
    a[0] = 0.0
    c[-1] = 0.0
    T = np.zeros((SIZE, SIZE), np.float64)
    idx = np.arange(SIZE)
    T[idx, idx] = b
    T[idx[1:], idx[1:] - 1] = a[1:]
    T[idx[:-1], idx[:-1] + 1] = c[:-1]
    return np.linalg.inv(T)


def _build_matrices(inputs):
    abx = np.asarray(inputs['alpha_base_x'], np.float32)
    atcx = np.asarray(inputs['alpha_time_coeff_x'], np.float32)
    atqx = np.asarray(inputs['alpha_time_quad_x'], np.float32)
    bby = np.asarray(inputs['beta_base_y'], np.float32)
    btcy = np.asarray(inputs['beta_time_coeff_y'], np.float32)
    btqy = np.asarray(inputs['beta_time_quad_y'], np.float32)

    SX = np.eye(SIZE)
    SY = np.eye(SIZE)
    t = 0.0
    for _ in range(NUM_STEPS):
        ax = _coeffs_at32(abx, atcx, atqx, t)
        SX = _solve_inv64(ax, DT / 2, DX) @ SX
        t += DT / 2
        by = _coeffs_at32(bby, btcy, btqy, t)
        SY = _solve_inv64(by, DT, DY) @ SY
        t += DT / 2
        ax = _coeffs_at32(abx, atcx, atqx, t)
        SX = _solve_inv64(ax, DT / 2, DX) @ SX
    return SX, SY


_NC_CACHE = {}


def _wait_cap(ins):
    """Max sync-wait slots codegen allows for this instruction."""
    tname = type(ins).__name__
    if tname in ('InstUnconditionalBranch', 'InstCompareAndBranch',
                 'InstExtSeq', 'InstBranchHint', 'InstSeqAssert'):
        return 10 ** 9
    if tname == 'InstMatmult':
        return 1
    outs = getattr(ins, 'outs', [])
    for o in outs:
        d = getattr(getattr(o, 'bass_ap', None), 'dtype', None) or getattr(o, 'dtype', None)
        if d is not None and 'float32r' in str(d):
            return 1
    if tname in ('InstActivation', 'InstTensorCopy', 'InstTensorTensor',
                 'InstTensorScalarPtr', 'InstTensorReduce'):
        return 1
    if tname == 'InstDMACopy':
        return 1
    return 3


def _fix_wait_limits(nc):
    """Post-scheduling pass: enforce per-instruction sync-wait-slot limits.

    Tile's add_semaphores emits waits that are minimal per-engine but not
    transitively minimal, and it does not know about the 1-slot limit of
    matmuls.  We (a) drop waits already implied transitively by the
    instruction's other waits / program order, and (b) move any remaining
    excess waits onto earlier same-engine instructions with free slots
    (always sound: the engine just stalls slightly earlier), checking the
    moved wait's producer does not depend on instructions between the new
    location and the original one.
    """
    import bass_rust  # noqa: F401

    prog = []  # (block, ins) in scheduled order
    for blk in nc.main_func.blocks:
        for ins in blk.instructions:
            prog.append(ins)

    # Per-sem cumulative update streams: sem_id -> list of (cum_value, prog_idx)
    sem_stream = {}
    # engine -> list of prog indices
    eng_stream = {}
    info = []  # per prog idx: dict(engine, waits, updates)
    for idx, ins in enumerate(prog):
        si = ins.sync_info
        eng = str(ins.engine)
        waits = list(si.on_wait) if si is not None else []
        updates = list(si.on_update) if si is not None else []
        for up in updates:
            lst = sem_stream.setdefault(up.id, [])
            prev = lst[-1][0] if lst else 0
            lst.append((prev + up.update_value, idx))
        eng_stream.setdefault(eng, []).append(idx)
        info.append({'engine': eng, 'waits': waits, 'updates': updates})

    def producer_of(sem_id, value):
        lst = sem_stream.get(sem_id, [])
        for cum, idx in lst:
            if cum >= value:
                return idx
        return None

    # Vector clocks: for each prog idx, observed sem floor map after its waits
    # resolve (and before its own updates).  vc_done[idx] includes own updates.
    vc = [None] * len(prog)
    vc_done = [None] * len(prog)
    prev_on_engine = {}
    prev_idx_map = {}
    for idx in range(len(prog)):
        eng = info[idx]['engine']
        base = {}
        p = prev_on_engine.get(eng)
        prev_idx_map[idx] = p
        if p is not None:
            base.update(vc_done[p])
        for w in info[idx]['waits']:
            base[w.id] = max(base.get(w.id, 0), w.wait_value)
            pr = producer_of(w.id, w.wait_value)
            if pr is not None and pr < idx:
                for k, v in vc_done[pr].items():
                    if v > base.get(k, 0):
                        base[k] = v
        vc[idx] = base
        done = dict(base)
        for up in info[idx]['updates']:
            # cumulative value after this instruction
            for cum, uidx in sem_stream[up.id]:
                if uidx == idx:
                    done[up.id] = max(done.get(up.id, 0), cum)
                    break
        vc_done[idx] = done
        prev_on_engine[eng] = idx

    n_moved = n_dropped = 0
    for idx, ins in enumerate(prog):
        cap = _wait_cap(ins)
        si = ins.sync_info
        if si is None:
            continue
        waits = list(si.on_wait)
        if len(waits) <= cap:
            continue
        eng = info[idx]['engine']
        p = prev_idx_map[idx]
        base = dict(vc_done[p]) if p is not None else {}

        # (a) drop transitively-implied waits
        kept = []
        for w in waits:
            other_floor = dict(base)
            for w2 in waits:
                if w2 is w:
                    continue
                pr = producer_of(w2.id, w2.wait_value)
                if pr is not None and pr < idx:
                    for k, v in vc_done[pr].items():
                        if v > other_floor.get(k, 0):
                            other_floor[k] = v
            if other_floor.get(w.id, 0) >= w.wait_value:
                n_dropped += 1
                continue
            kept.append(w)
        waits = kept

        # (b) move excess to earlier same-engine instructions
        if len(waits) > cap:
            own_sems = {up.id for j in eng_stream[eng] for up in info[j]['updates']}
            estream = eng_stream[eng]
            my_pos = estream.index(idx)
            excess = waits[:-cap] if cap else waits
            waits = waits[len(excess):]
            for w in excess:
                pr = producer_of(w.id, w.wait_value)
                placed = False
                for back in range(my_pos - 1, -1, -1):
                    tgt = estream[back]
                    tins = prog[tgt]
                    if type(tins).__name__ not in (
                            'InstMatmult', 'InstActivation', 'InstTensorCopy',
                            'InstDMACopy', 'InstTensorTensor', 'InstMemset',
                            'InstDrain', 'InstEventSemaphore', 'InstNoOp'):
                        continue
                    tsi = tins.sync_info
                    t_waits = list(tsi.on_wait) if tsi is not None else []
                    if len(t_waits) >= _wait_cap(tins):
                        continue
                    # safety: producer of w must not depend on this engine at or
                    # after tgt
                    if pr is not None:
                        dep = vc_done[pr]
                        ok = True
                        for sid in own_sems:
                            need = dep.get(sid, 0)
                            if need:
                                pidx = producer_of(sid, need)
                                if pidx is not None and pidx >= tgt:
                                    ok = False
                                    break
                        if not ok:
                            continue
                    t_waits.append(w)
                    import bass_rust as _br
                    t_upd = list(tsi.on_update) if tsi is not None else []
                    tins.sync_info = _br.SyncInfo(on_wait=t_waits, on_update=t_upd)
                    # update bookkeeping so later decisions see it
                    info[tgt]['waits'] = t_waits
                    placed = True
                    n_moved += 1
                    break
                if not placed:
                    raise RuntimeError(
                        f"could not relocate wait {w} from {ins.name}")
        ins.sync_info = type(si)(on_wait=waits, on_update=list(si.on_update))
        info[idx]['waits'] = waits
    return n_dropped, n_moved


def _build_nc(repeat=None):
    key = ('nc', repeat)
    if key in _NC_CACHE:
        return _NC_CACHE[key]
    nc = bass.Bass()
    # Single input blob = [SY^T (2 tiles) | SX^T (2) | u-shard (32)], fp16,
    # partition-major in DRAM so each chunk's DMA descriptors are multi-KB
    # contiguous runs per partition.
    # Flat per-partition blob: 4 band-packed matrix windows (only the
    # BAND-wide column windows the matmuls actually read are shipped) then
    # the 32 u tiles.  [p, free] layout keeps every DMA descriptor one
    # contiguous multi-KB run per partition.
    blob = nc.dram_tensor("blob", [P, FREE], F16, kind="ExternalInput")
    out = nc.dram_tensor("out", [P * G, SIZE], F16, kind="ExternalOutput")

    bv = blob
    outv = out.rearrange("(p g) w -> p g w", g=G)

    with tile.TileContext(nc) as tc:
        with (
            tc.tile_pool(name="blobp", bufs=1) as bpool,
            tc.tile_pool(name="t1", bufs=4) as t1pool,
            tc.tile_pool(name="opool", bufs=1) as opool,
            tc.tile_pool(name="ps", bufs=PSUM_BUFS, space="PSUM") as pspool,
        ):
            import contextlib
            loop_ctx = tc.For_i(0, repeat, 1) if repeat else contextlib.nullcontext()
            loop_ctx.__enter__()

            bsb = bpool.tile([P, FREE], F16, tag="blob")
            # chunk 0 carries the matrix windows + the first image PAIR
            # (group 0's full working set) so the pipeline starts as soon as
            # it lands; the rest streams in 2-group chunks behind it on the
            # same SP queue.
            # chunk 0 is split across BOTH HWDGE queues (SP: matrix windows
            # + image 0, ACT: image 1) so group 0's working set lands ~35%
            # sooner; ACT's queue is otherwise idle until the first copy.
            c0a = MATF + 2 * SIZE
            c0 = MATF + 4 * SIZE
            nc.sync.dma_start(out=bsb[:, 0:c0a], in_=bv[:, 0:c0a])
            nc.scalar.dma_start(out=bsb[:, c0a:c0], in_=bv[:, c0a:c0])
            spans = [(c0, c0 + 1024)] + [
                (c0 + 1024 + 2048 * k, c0 + 1024 + 2048 * (k + 1))
                for k in range(3)]
            assert spans[-1][1] == FREE
            for e0, e1 in spans:
                nc.sync.dma_start(out=bsb[:, e0:e1], in_=bv[:, e0:e1])

            syt_w = (bsb[:, 0:WB], bsb[:, WB:2 * WB])
            sxt_w = (bsb[:, 2 * WB:3 * WB], bsb[:, 3 * WB:4 * WB])

            def ug(b, kh):
                off = MATF + SIZE * (2 * b + kh)
                return bsb[:, off:off + SIZE]

            # SY/SX decay geometrically off the diagonal (coeff <= ~5e-3 per
            # step), so entries with |i-j| > BAND are < 1e-9 and each k-tile
            # only contributes to output columns within its banded window.
            # Per-element has_written semantics merge the two k-tile column
            # ranges: the overlap accumulates, the rest overwrites.
            n0w = slice(0, P + BAND)
            n1w = slice(P - BAND, SIZE)

            def _emit_banded(nc, ps, m, lhs_of, rhs_w):
                nc.tensor.matmul(ps[:, m, n0w], lhsT=lhs_of(0),
                                 rhs=rhs_w[0], start=True, stop=False)
                nc.tensor.matmul(ps[:, m, n1w], lhsT=lhs_of(1),
                                 rhs=rhs_w[1], start=False, stop=True)

            # Process images in groups of 2: both images' MM1 accumulate into
            # one 2-bank PSUM tile, evacuated by a single 1024-elem copy
            # (halves the per-instruction overhead of the PSUM-evacuation
            # copies, which only ACT and DVE can perform).  The PE emission
            # is software-pipelined (MM1 of group g+1 before MM2 of group g)
            # so the PE never sits behind the in-flight t1 copy in its own
            # program order.  Blocks of 2 groups share one output tile; the
            # t1 copies of a block go to one engine and the ot copies to the
            # other (alternating per block) so each block's output DMA waits
            # on a single cumulative semaphore.  Output DMAs ride the SWDGE
            # queue of the otherwise-idle Pool engine, so output packets
            # never queue FIFO behind input packets.
            # Image groups: 8 pairs.  (Measured: splitting the tail into
            # single-image groups adds copy-instruction overhead to the
            # capacity-bound ACT/DVE phase and lengthens the dependency
            # tail — pairs throughout is faster.)
            groups = [(2 * g, 2) for g in range(8)]

            # Both stages of a group share ONE 2-bank PSUM tile: MM2
            # overwrites the banks MM1 used.  MM2's wait-after-read on the
            # t1 evacuation IS the existing chain dependency, so the shared
            # tile adds no serialization — but halving the per-group PSUM
            # footprint doubles the pool depth to 4 in-flight groups, which
            # takes the group-recycle chain (MM1 -> ca -> MM2 -> cb ->
            # MM1(g+4)) off the critical path: the phase runs at ACT/DVE
            # copy capacity instead.
            def mm1(gi, b0, n):
                t1d = t1pool.tile([P, 4, SIZE], F16, tag="t1d")
                ps = pspool.tile([P, 4, SIZE], F32, tag="ps")
                for i in range(n):
                    for m in range(2):
                        ms = slice(m * P, (m + 1) * P)
                        _emit_banded(nc, ps, 2 * i + m,
                                     lambda kh: ug(b0 + i, kh)[:, ms], syt_w)
                # Dedicated copy streams: ALL t1 evacuations on DVE, all ot
                # evacuations on ACT.  The supply-paced ca stream then never
                # queues behind a chain-lagged cb on the same engine (the
                # ping-pong head-of-line blocking that per-group alternation
                # suffers), and the faster ACT engine drains the lagging ot
                # stream.
                nc.vector.tensor_copy(out=t1d[:, 0:2 * n, :],
                                      in_=ps[:, 0:2 * n, :])
                return t1d, ps

            def mm2(gi, b0, n, t1d, ps):
                ot = opool.tile([P, 2 * n, SIZE], F16, tag=f"ot{gi}")
                for i in range(n):
                    for m in range(2):
                        ms = slice(m * P, (m + 1) * P)
                        _emit_banded(nc, ps, 2 * i + m,
                                     lambda kw: t1d[:, 2 * i + kw, ms], sxt_w)
                nc.scalar.copy(out=ot[:], in_=ps[:, 0:2 * n, :])
                # one output DMA per group from SP (queue free of input by
                # the time outputs start; HWDGE keeps the final drain short);
                # waits only on cb's single cumulative semaphore.  The final
                # group's DMA is issued by ACT itself right after its own cb
                # (program order — skips the cross-engine semaphore hop on
                # the critical tail).
                deng = nc.scalar if gi == len(groups) - 1 else nc.sync
                deng.dma_start(out=outv[:, 2 * b0:2 * b0 + 2 * n, :],
                               in_=ot[:])

            # prologue depth 1 (MM1 of g+1 ahead of MM2 of g).  Measured:
            # depth 2 delays the MM2s enough that the ot evacuations lag and
            # the tail grows — depth 1 is optimal even with the 4-deep
            # shared-PSUM pool.
            DEPTH = 1
            pending = []
            for gi, (b0, n) in enumerate(groups):
                t1d, ps = mm1(gi, b0, n)
                pending.append((gi, b0, n, t1d, ps))
                if len(pending) > DEPTH:
                    mm2(*pending.pop(0))
            for args in pending:
                mm2(*args)

            loop_ctx.__exit__(None, None, None)

    _fix_wait_limits(nc)
    _NC_CACHE[key] = nc
    return nc


WB = P + BAND   # banded rhs window width (matches n0w/n1w in _build_nc)
MATF = 4 * WB
FREE = MATF + G * SIZE


def _make_blob(syt16, sxt16, shard16):
    # flat per-partition layout: 4 band-packed matrix windows, then the
    # 32 u tiles ([p, free]; matches _build_nc's blob geometry)
    arr = np.zeros((P, FREE), np.float16)
    arr[:, 0:WB] = syt16[0:P, 0:WB]
    arr[:, WB:2 * WB] = syt16[P:SIZE, P - BAND:SIZE]
    arr[:, 2 * WB:3 * WB] = sxt16[0:P, 0:WB]
    arr[:, 3 * WB:4 * WB] = sxt16[P:SIZE, P - BAND:SIZE]
    arr[:, MATF:] = shard16.reshape(B_PER, 2, P, SIZE).transpose(
        2, 0, 1, 3).reshape(P, G * SIZE)
    return np.ascontiguousarray(arr)


def kernel(**inputs):
    u = np.asarray(inputs['u'], np.float32).reshape(B_FULL, SIZE, SIZE)
    SX, SY = _build_matrices(inputs)
    syt16 = SY.T.astype(np.float16)
    sxt16 = SX.T.astype(np.float16)
    u16 = u.astype(np.float16)

    nc = _build_nc()
    in_maps = []
    for c in range(N_CORES):
        shard = u16[c * B_PER:(c + 1) * B_PER]  # [16, 256, 256]
        in_maps.append({'blob': _make_blob(syt16, sxt16, shard)})

    res = run_bass_kernel_spmd(nc, in_maps, core_ids=list(range(N_CORES)))
    global LAST_EXEC_NS
    LAST_EXEC_NS = res.exec_time_ns
    outs = []
    for r in res.results:
        # [P, G, SIZE] -> g = 2b + kh, image h = kh*128 + p
        o = r['out'].reshape(P, B_PER, 2, SIZE).transpose(1, 2, 0, 3)
        outs.append(o.reshape(B_PER, SIZE, SIZE))
    full = np.concatenate(outs, axis=0).reshape(B_FULL, 1, SIZE, SIZE)
    return full.astype(np.float32)


LAST_EXEC_NS = None


# revision 51
# speedup vs baseline: 1.0137x; 1.0137x over previous
"""Trainium2 Bass kernel for the ADI diffusion layer.

The reference applies 10 ADI time steps to u[B=128, 1, 256, 256]; each step
does three tridiagonal (Thomas) solves along W or H with coefficients that
depend only on tiny [256] parameter vectors and the (compile-time-known)
step times.  The whole network is linear in u, and the x-axis solves
(right-multiplications) commute with the y-axis solves (left-
multiplications), so the entire computation collapses to

    out[b] = SY @ u[b] @ SX^T

with SX = product of the 20 x-solve inverses and SY = product of the 10
y-solve inverses, both 256x256, precomputed on host in float64 from the
parameter vectors.

On-device work per core (batch sharded 8 ways, 16 images/core):
  MM1: T1t = (SY @ u_b)^T  via  matmul(lhsT=u_b-tile, rhs=SY^T)
  MM2: out_b = T1t^T @ SX^T via matmul(lhsT=T1t-tile, rhs=SX^T)
Both stages contract on the partition dimension with the data tile as the
stationary operand, so the output lands in natural layout with zero
transposes.  The kernel is memory-bound: everything (u, SY^T, SX^T, the
intermediate, and the output) is carried in float16, which keeps the
end-to-end relative error ~4e-4 (measured) while halving HBM traffic vs
fp32 and running the PE at 1 cycle/row instead of 4.

SX and SY decay geometrically off the diagonal (per-step coeff <= ~5e-3),
so each 128-row contraction tile only feeds output columns within BAND of
its own index range ('banded2' matmuls: the overlap region accumulates via
per-element PSUM has_written, the rest overwrites; HW-verified).

DRAM layouts are partition-major/flat ([p, free]) so every DMA descriptor
covers a multi-KB contiguous run per partition; the matrices ship only
their band-packed 136-column windows.  Images are processed in groups of 2;
both stages of a group SHARE one 2-bank PSUM tile (MM2 overwrites the banks
MM1 used — its wait-after-read on the t1 evacuation is the existing chain
dependency, so sharing adds no serialization), which halves the per-group
PSUM footprint and lets the single-tag pool hold 4 in-flight groups in the
8 banks.  That takes the group-recycle chain off the critical path: the
middle phase runs at the ACT/DVE PSUM-evacuation capacity (single 1024-elem
copies, the largest the banks allow, on DEDICATED streams: all t1 copies on
DVE, all ot copies on ACT, so the supply-paced t1 stream never queues
behind a chain-lagged ot copy on the same engine).  The
PE emission is software-pipelined (MM1 of group g+1 before MM2 of group g;
measured: deeper prologues delay the ot evacuations and grow the tail).  Chunk 0 (matrices +
group 0) is split across the SP and ACT HWDGE queues so compute starts
~0.6us earlier; the rest streams sequentially on SP (packet dispatch is a
global ~90/us, so sequential FIFO on one queue is optimal for a linear
consumer).  Outputs ride the SP queue (free of input by the time outputs
start; HWDGE keeps the postamble DGE drain short), except the final
group's DMA which ACT issues right after its own evacuation copy to skip
a cross-engine semaphore hop on the critical tail.

Walrus enforces tiny sync-wait-slot budgets (1 for matmuls, ACT/DVE copies
and DMACopies) that Tile's scheduler does not know about;
_fix_wait_limits() post-processes the scheduled BIR to drop transitively
implied waits and relocate the rest onto earlier same-engine instructions.
"""

import numpy as np

import concourse.bass as bass
import concourse.mybir as mybir
import concourse.tile as tile
from concourse.bass_utils import run_bass_kernel_spmd

SIZE = 256
B_FULL = 128
N_CORES = 8
B_PER = B_FULL // N_CORES  # 16 images per core
G = B_PER * 2              # 32 [128, 256] partition-tiles of u per core
P = 128

DT = 0.01
DX = 1.0
DY = 1.0
NUM_STEPS = 10
EPS = 1e-6

F32 = mybir.dt.float32
F16 = mybir.dt.float16
BAND = 8
PSUM_BUFS = 4


def _smooth32(v):
    vp = np.concatenate([v[:1], v, v[-1:]]).astype(np.float32)
    return (np.float32(0.25) * vp[:-2] + np.float32(0.5) * vp[1:-1]
            + np.float32(0.25) * vp[2:]).astype(np.float32)


def _coeffs_at32(base, lin, quad, t):
    t = np.float32(t)
    return np.maximum(base + lin * t + quad * (t * t), np.float32(EPS)).astype(np.float32)


def _solve_inv64(alpha_vec32, dt, dh):
    """Inverse of the tridiagonal system the reference's _diffuse solves.

    Coefficient construction mirrors the reference in float32; the inverse
    itself is taken in float64.
    """
    coeff = (_smooth32(alpha_vec32) * np.float32(dt) / np.float32(dh * dh)).astype(np.float32)
    a = (-coeff).astype(np.float64)
    c = (-coeff).astype(np.float64)
    b = (np.float32(1.0) + np.float32(2.0) * coeff).astype(np.float32).astype(np.float64)
    b[0] = np.float64(np.float32(1.0) + coeff[0])
    b[-1] = np.float64(np.float32(1.0) + coeff[-1])
    a[0] = 0.0
    c[-1] = 0.0
    T = np.zeros((SIZE, SIZE), np.float64)
    idx = np.arange(SIZE)
    T[idx, idx] = b
    T[idx[1:], idx[1:] - 1] = a[1:]
    T[idx[:-1], idx[:-1] + 1] = c[:-1]
    return np.linalg.inv(T)


def _build_matrices(inputs):
    abx = np.asarray(inputs['alpha_base_x'], np.float32)
    atcx = np.asarray(inputs['alpha_time_coeff_x'], np.float32)
    atqx = np.asarray(inputs['alpha_time_quad_x'], np.float32)
    bby = np.asarray(inputs['beta_base_y'], np.float32)
    btcy = np.asarray(inputs['beta_time_coeff_y'], np.float32)
    btqy = np.asarray(inputs['beta_time_quad_y'], np.float32)

    SX = np.eye(SIZE)
    SY = np.eye(SIZE)
    t = 0.0
    for _ in range(NUM_STEPS):
        ax = _coeffs_at32(abx, atcx, atqx, t)
        SX = _solve_inv64(ax, DT / 2, DX) @ SX
        t += DT / 2
        by = _coeffs_at32(bby, btcy, btqy, t)
        SY = _solve_inv64(by, DT, DY) @ SY
        t += DT / 2
        ax = _coeffs_at32(abx, atcx, atqx, t)
        SX = _solve_inv64(ax, DT / 2, DX) @ SX
    return SX, SY


_NC_CACHE = {}


def _wait_cap(ins):
    """Max sync-wait slots codegen allows for this instruction."""
    tname = type(ins).__name__
    if tname in ('InstUnconditionalBranch', 'InstCompareAndBranch',
                 'InstExtSeq', 'InstBranchHint', 'InstSeqAssert'):
        return 10 ** 9
    if tname == 'InstMatmult':
        return 1
    outs = getattr(ins, 'outs', [])
    for o in outs:
        d = getattr(getattr(o, 'bass_ap', None), 'dtype', None) or getattr(o, 'dtype', None)
        if d is not None and 'float32r' in str(d):
            return 1
    if tname in ('InstActivation', 'InstTensorCopy', 'InstTensorTensor',
                 'InstTensorScalarPtr', 'InstTensorReduce'):
        return 1
    if tname == 'InstDMACopy':
        return 1
    return 3


def _fix_wait_limits(nc):
    """Post-scheduling pass: enforce per-instruction sync-wait-slot limits.

    Tile's add_semaphores emits waits that are minimal per-engine but not
    transitively minimal, and it does not know about the 1-slot limit of
    matmuls.  We (a) drop waits already implied transitively by the
    instruction's other waits / program order, and (b) move any remaining
    excess waits onto earlier same-engine instructions with free slots
    (always sound: the engine just stalls slightly earlier), checking the
    moved wait's producer does not depend on instructions between the new
    location and the original one.
    """
    import bass_rust  # noqa: F401

    prog = []  # (block, ins) in scheduled order
    for blk in nc.main_func.blocks:
        for ins in blk.instructions:
            prog.append(ins)

    # Per-sem cumulative update streams: sem_id -> list of (cum_value, prog_idx)
    sem_stream = {}
    # engine -> list of prog indices
    eng_stream = {}
    info = []  # per prog idx: dict(engine, waits, updates)
    for idx, ins in enumerate(prog):
        si = ins.sync_info
        eng = str(ins.engine)
        waits = list(si.on_wait) if si is not None else []
        updates = list(si.on_update) if si is not None else []
        for up in updates:
            lst = sem_stream.setdefault(up.id, [])
            prev = lst[-1][0] if lst else 0
            lst.append((prev + up.update_value, idx))
        eng_stream.setdefault(eng, []).append(idx)
        info.append({'engine': eng, 'waits': waits, 'updates': updates})

    def producer_of(sem_id, value):
        lst = sem_stream.get(sem_id, [])
        for cum, idx in lst:
            if cum >= value:
                return idx
        return None

    # Vector clocks: for each prog idx, observed sem floor map after its waits
    # resolve (and before its own updates).  vc_done[idx] includes own updates.
    vc = [None] * len(prog)
    vc_done = [None] * len(prog)
    prev_on_engine = {}
    prev_idx_map = {}
    for idx in range(len(prog)):
        eng = info[idx]['engine']
        base = {}
        p = prev_on_engine.get(eng)
        prev_idx_map[idx] = p
        if p is not None:
            base.update(vc_done[p])
        for w in info[idx]['waits']:
            base[w.id] = max(base.get(w.id, 0), w.wait_value)
            pr = producer_of(w.id, w.wait_value)
            if pr is not None and pr < idx:
                for k, v in vc_done[pr].items():
                    if v > base.get(k, 0):
                        base[k] = v
        vc[idx] = base
        done = dict(base)
        for up in info[idx]['updates']:
            # cumulative value after this instruction
            for cum, uidx in sem_stream[up.id]:
                if uidx == idx:
                    done[up.id] = max(done.get(up.id, 0), cum)
                    break
        vc_done[idx] = done
        prev_on_engine[eng] = idx

    n_moved = n_dropped = 0
    for idx, ins in enumerate(prog):
        cap = _wait_cap(ins)
        si = ins.sync_info
        if si is None:
            continue
        waits = list(si.on_wait)
        if len(waits) <= cap:
            continue
        eng = info[idx]['engine']
        p = prev_idx_map[idx]
        base = dict(vc_done[p]) if p is not None else {}

        # (a) drop transitively-implied waits
        kept = []
        for w in waits:
            other_floor = dict(base)
            for w2 in waits:
                if w2 is w:
                    continue
                pr = producer_of(w2.id, w2.wait_value)
                if pr is not None and pr < idx:
                    for k, v in vc_done[pr].items():
                        if v > other_floor.get(k, 0):
                            other_floor[k] = v
            if other_floor.get(w.id, 0) >= w.wait_value:
                n_dropped += 1
                continue
            kept.append(w)
        waits = kept

        # (b) move excess to earlier same-engine instructions
        if len(waits) > cap:
            own_sems = {up.id for j in eng_stream[eng] for up in info[j]['updates']}
            estream = eng_stream[eng]
            my_pos = estream.index(idx)
            excess = waits[:-cap] if cap else waits
            waits = waits[len(excess):]
            for w in excess:
                pr = producer_of(w.id, w.wait_value)
                placed = False
                for back in range(my_pos - 1, -1, -1):
                    tgt = estream[back]
                    tins = prog[tgt]
                    if type(tins).__name__ not in (
                            'InstMatmult', 'InstActivation', 'InstTensorCopy',
                            'InstDMACopy', 'InstTensorTensor', 'InstMemset',
                            'InstDrain', 'InstEventSemaphore', 'InstNoOp'):
                        continue
                    tsi = tins.sync_info
                    t_waits = list(tsi.on_wait) if tsi is not None else []
                    if len(t_waits) >= _wait_cap(tins):
                        continue
                    # safety: producer of w must not depend on this engine at or
                    # after tgt
                    if pr is not None:
                        dep = vc_done[pr]
                        ok = True
                        for sid in own_sems:
                            need = dep.get(sid, 0)
                            if need:
                                pidx = producer_of(sid, need)
                                if pidx is not None and pidx >= tgt:
                                    ok = False
                                    break
                        if not ok:
                            continue
                    t_waits.append(w)
                    import bass_rust as _br
                    t_upd = list(tsi.on_update) if tsi is not None else []
                    tins.sync_info = _br.SyncInfo(on_wait=t_waits, on_update=t_upd)
                    # update bookkeeping so later decisions see it
                    info[tgt]['waits'] = t_waits
                    placed = True
                    n_moved += 1
                    break
                if not placed:
                    raise RuntimeError(
                        f"could not relocate wait {w} from {ins.name}")
        ins.sync_info = type(si)(on_wait=waits, on_update=list(si.on_update))
        info[idx]['waits'] = waits
    return n_dropped, n_moved


def _build_nc(repeat=None):
    key = ('nc', repeat)
    if key in _NC_CACHE:
        return _NC_CACHE[key]
    nc = bass.Bass()
    # Single input blob = [SY^T (2 tiles) | SX^T (2) | u-shard (32)], fp16,
    # partition-major in DRAM so each chunk's DMA descriptors are multi-KB
    # contiguous runs per partition.
    # Flat per-partition blob: 4 band-packed matrix windows (only the
    # BAND-wide column windows the matmuls actually read are shipped) then
    # the 32 u tiles.  [p, free] layout keeps every DMA descriptor one
    # contiguous multi-KB run per partition.
    blob = nc.dram_tensor("blob", [P, FREE], F16, kind="ExternalInput")
    out = nc.dram_tensor("out", [P * G, SIZE], F16, kind="ExternalOutput")

    bv = blob
    outv = out.rearrange("(p g) w -> p g w", g=G)

    with tile.TileContext(nc) as tc:
        with (
            tc.tile_pool(name="blobp", bufs=1) as bpool,
            tc.tile_pool(name="t1", bufs=4) as t1pool,
            tc.tile_pool(name="opool", bufs=1) as opool,
            tc.tile_pool(name="ps", bufs=PSUM_BUFS, space="PSUM") as pspool,
        ):
            import contextlib
            loop_ctx = tc.For_i(0, repeat, 1) if repeat else contextlib.nullcontext()
            loop_ctx.__enter__()

            bsb = bpool.tile([P, FREE], F16, tag="blob")
            # chunk 0 carries the matrix windows + the first image PAIR
            # (group 0's full working set) so the pipeline starts as soon as
            # it lands; the rest streams in 2-group chunks behind it on the
            # same SP queue.
            # chunk 0 is split across BOTH HWDGE queues (SP: matrix windows
            # + image 0, ACT: image 1) so group 0's working set lands ~35%
            # sooner; ACT's queue is otherwise idle until the first copy.
            c0a = MATF + 2 * SIZE
            c0 = MATF + 4 * SIZE
            nc.sync.dma_start(out=bsb[:, 0:c0a], in_=bv[:, 0:c0a])
            nc.scalar.dma_start(out=bsb[:, c0a:c0], in_=bv[:, c0a:c0])
            spans = [(c0, c0 + 1024)] + [
                (c0 + 1024 + 2048 * k, c0 + 1024 + 2048 * (k + 1))
                for k in range(3)]
            assert spans[-1][1] == FREE
            for e0, e1 in spans:
                nc.sync.dma_start(out=bsb[:, e0:e1], in_=bv[:, e0:e1])

            syt_w = (bsb[:, 0:WB], bsb[:, WB:2 * WB])
            sxt_w = (bsb[:, 2 * WB:3 * WB], bsb[:, 3 * WB:4 * WB])

            def ug(b, kh):
                off = MATF + SIZE * (2 * b + kh)
                return bsb[:, off:off + SIZE]

            # SY/SX decay geometrically off the diagonal (coeff <= ~5e-3 per
            # step), so entries with |i-j| > BAND are < 1e-9 and each k-tile
            # only contributes to output columns within its banded window.
            # Per-element has_written semantics merge the two k-tile column
            # ranges: the overlap accumulates, the rest overwrites.
            n0w = slice(0, P + BAND)
            n1w = slice(P - BAND, SIZE)

            def _emit_banded(nc, ps, m, lhs_of, rhs_w):
                nc.tensor.matmul(ps[:, m, n0w], lhsT=lhs_of(0),
                                 rhs=rhs_w[0], start=True, stop=False)
                nc.tensor.matmul(ps[:, m, n1w], lhsT=lhs_of(1),
                                 rhs=rhs_w[1], start=False, stop=True)

            # Process images in groups of 2: both images' MM1 accumulate into
            # one 2-bank PSUM tile, evacuated by a single 1024-elem copy
            # (halves the per-instruction overhead of the PSUM-evacuation
            # copies, which only ACT and DVE can perform).  The PE emission
            # is software-pipelined (MM1 of group g+1 before MM2 of group g)
            # so the PE never sits behind the in-flight t1 copy in its own
            # program order.  Blocks of 2 groups share one output tile; the
            # t1 copies of a block go to one engine and the ot copies to the
            # other (alternating per block) so each block's output DMA waits
            # on a single cumulative semaphore.  Output DMAs ride the SWDGE
            # queue of the otherwise-idle Pool engine, so output packets
            # never queue FIFO behind input packets.
            # Image groups: 8 pairs.  (Measured: splitting the tail into
            # single-image groups adds copy-instruction overhead to the
            # capacity-bound ACT/DVE phase and lengthens the dependency
            # tail — pairs throughout is faster.)
            groups = [(2 * g, 2) for g in range(8)]

            # Both stages of a group share ONE 2-bank PSUM tile: MM2
            # overwrites the banks MM1 used.  MM2's wait-after-read on the
            # t1 evacuation IS the existing chain dependency, so the shared
            # tile adds no serialization — but halving the per-group PSUM
            # footprint doubles the pool depth to 4 in-flight groups, which
            # takes the group-recycle chain (MM1 -> ca -> MM2 -> cb ->
            # MM1(g+4)) off the critical path: the phase runs at ACT/DVE
            # copy capacity instead.
            def mm1(gi, b0, n):
                t1d = t1pool.tile([P, 4, SIZE], F16, tag="t1d")
                ps = pspool.tile([P, 4, SIZE], F32, tag="ps")
                for i in range(n):
                    for m in range(2):
                        ms = slice(m * P, (m + 1) * P)
                        _emit_banded(nc, ps, 2 * i + m,
                                     lambda kh: ug(b0 + i, kh)[:, ms], syt_w)
                # Dedicated copy streams: t1 evacuations on DVE, ot
                # evacuations on ACT.  The supply-paced ca stream then never
                # queues behind a chain-lagged cb on the same engine (the
                # ping-pong head-of-line blocking that per-group alternation
                # suffers), and the faster ACT engine drains the lagging ot
                # stream.  Exception: ca0 runs on ACT — ACT is idle before
                # its first chain-ready ot copy anyway, and taking one copy
                # off DVE pulls the whole t1 stream (and everything chained
                # behind it) earlier.
                ca = nc.scalar.copy if gi == 0 else nc.vector.tensor_copy
                ca(out=t1d[:, 0:2 * n, :], in_=ps[:, 0:2 * n, :])
                return t1d, ps

            def mm2(gi, b0, n, t1d, ps):
                ot = opool.tile([P, 2 * n, SIZE], F16, tag=f"ot{gi}")
                for i in range(n):
                    for m in range(2):
                        ms = slice(m * P, (m + 1) * P)
                        _emit_banded(nc, ps, 2 * i + m,
                                     lambda kw: t1d[:, 2 * i + kw, ms], sxt_w)
                if gi == len(groups) - 1:
                    # tail: both engines are free by now and the halves have
                    # a single PE producer each — split the final evacuation
                    # to halve the last copy's latency.
                    nc.scalar.copy(out=ot[:, 0:n, :], in_=ps[:, 0:n, :])
                    nc.vector.tensor_copy(out=ot[:, n:2 * n, :],
                                          in_=ps[:, n:2 * n, :])
                else:
                    nc.scalar.copy(out=ot[:], in_=ps[:, 0:2 * n, :])
                # one output DMA per group from SP (queue free of input by
                # the time outputs start; HWDGE keeps the final drain short);
                # waits only on cb's single cumulative semaphore.  The final
                # group's DMA is issued by ACT itself right after its own cb
                # (program order — skips the cross-engine semaphore hop on
                # the critical tail).
                deng = nc.scalar if gi == len(groups) - 1 else nc.sync
                deng.dma_start(out=outv[:, 2 * b0:2 * b0 + 2 * n, :],
                               in_=ot[:])

            # prologue depth 1 (MM1 of g+1 ahead of MM2 of g).  Measured:
            # depth 2 delays the MM2s enough that the ot evacuations lag and
            # the tail grows — depth 1 is optimal even with the 4-deep
            # shared-PSUM pool.
            DEPTH = 1
            pending = []
            for gi, (b0, n) in enumerate(groups):
                t1d, ps = mm1(gi, b0, n)
                pending.append((gi, b0, n, t1d, ps))
                if len(pending) > DEPTH:
                    mm2(*pending.pop(0))
            for args in pending:
                mm2(*args)

            loop_ctx.__exit__(None, None, None)

    _fix_wait_limits(nc)
    _NC_CACHE[key] = nc
    return nc


WB = P + BAND   # banded rhs window width (matches n0w/n1w in _build_nc)
MATF = 4 * WB
FREE = MATF + G * SIZE


def _make_blob(syt16, sxt16, shard16):
    # flat per-partition layout: 4 band-packed matrix windows, then the
    # 32 u tiles ([p, free]; matches _build_nc's blob geometry)
    arr = np.zeros((P, FREE), np.float16)
    arr[:, 0:WB] = syt16[0:P, 0:WB]
    arr[:, WB:2 * WB] = syt16[P:SIZE, P - BAND:SIZE]
    arr[:, 2 * WB:3 * WB] = sxt16[0:P, 0:WB]
    arr[:, 3 * WB:4 * WB] = sxt16[P:SIZE, P - BAND:SIZE]
    arr[:, MATF:] = shard16.reshape(B_PER, 2, P, SIZE).transpose(
        2, 0, 1, 3).reshape(P, G * SIZE)
    return np.ascontiguousarray(arr)


def kernel(**inputs):
    u = np.asarray(inputs['u'], np.float32).reshape(B_FULL, SIZE, SIZE)
    SX, SY = _build_matrices(inputs)
    syt16 = SY.T.astype(np.float16)
    sxt16 = SX.T.astype(np.float16)
    u16 = u.astype(np.float16)

    nc = _build_nc()
    in_maps = []
    for c in range(N_CORES):
        shard = u16[c * B_PER:(c + 1) * B_PER]  # [16, 256, 256]
        in_maps.append({'blob': _make_blob(syt16, sxt16, shard)})

    res = run_bass_kernel_spmd(nc, in_maps, core_ids=list(range(N_CORES)))
    global LAST_EXEC_NS
    LAST_EXEC_NS = res.exec_time_ns
    outs = []
    for r in res.results:
        # [P, G, SIZE] -> g = 2b + kh, image h = kh*128 + p
        o = r['out'].reshape(P, B_PER, 2, SIZE).transpose(1, 2, 0, 3)
        outs.append(o.reshape(B_PER, SIZE, SIZE))
    full = np.concatenate(outs, axis=0).reshape(B_FULL, 1, SIZE, SIZE)
    return full.astype(np.float32)


LAST_EXEC_NS = None
